# revision 49
# baseline (speedup 1.0000x reference)
"""Trainium2 Bass kernel for multi-head attention + output projection.

Problem: B=4, N=2048, D=512, H=8 heads (head_dim 64), TEMP=8.0.
  logits = (Q @ K^T) / TEMP per head; P = softmax(logits); out = P @ V
  final = concat_heads(out) @ W_comb.T + b_comb

Sharding: 8 cores = 4 batches x 2 query-halves. Each core computes a full
(1024, 512) output slab independently (keys/values replicated per batch);
no collectives. Gather = pure reshape on host. Q, K and W are passed to
each core PRE-TRANSPOSED (d-major); V is passed host-packed as V_ext
(per-head [128, 16, 65] tiles with the softmax-denominator ones column
baked in at col 64) so each head is ONE contiguous-per-partition DMA.

Per-core algorithm ("transposed attention", so the PV matmul needs no
transpose of the softmax matrix):
  S^T[k, q] = K_h @ Q_h^T   (bf16 Q/K; head pair packs the 128
              contraction rows -> row-tiled matmuls at partitions 0/64)
  E^T = exp(S^T / TEMP)     (ScalarE straight from PSUM, f32r out; no
              max-subtraction: logits ~ N(0,1), exp is fp32-safe)
  O^T_ext = V_ext^T @ E^T   (f32r; stationary = V tile with a ones column
              at index 64, so partition 64 of the accumulator becomes the
              softmax denominator)
  O = O^T / denom           (reciprocals of the PSUM denominator rows on
              DVE into a [33, 1024] strip tile, partition-broadcast via a
              selector matmul through PE, column-chunked in-place
              tensor_muls -- no DRAM round trips)
  F += [O_A^T; O_B^T].T @ [W_A^T; W_B^T]  (pair-packed: ONE matmul with
              128-deep contraction per q-tile into [128, 2, 512] PSUM
              tiles, two q-tiles per drain; bias folded into the first
              pair's add; fsb accumulates in f32r)
The last pair folds fsb into its projection PSUM group via an identity
matmul, drains through ACT+DVE copies into bf16 staging, and ships the
output as bf16 (host converts back to f32; ~2.9e-3 total rel err).

Scheduling notes: ACT (the exp stream, ~133us busy of the ~150us total)
is the bottleneck engine; everything else hides behind it. The S matmuls
carry high priority so the exp stream never starves behind PV catch-up.
The projection of pair p-1 borrows the o-accumulator PSUM slot rotation
([o65, o97, rbp, proj x4] per pair) while pair p's PV naturally defers
(the 16-deep e-tile pool covers the lag). Warm-up matmuls keep the PE
p-state ramped at the start.
"""

import ml_dtypes
import numpy as np

import concourse.bass as bass
import concourse.mybir as mybir
from concourse.tile import TileContext

F32 = mybir.dt.float32
F32R = mybir.dt.float32r
BF16 = mybir.dt.bfloat16

B, N, D, H = 4, 2048, 512, 8
HEAD = 64
TEMP = 8.0
NQ = N // 2          # queries per core
NCORES = 8
NKT = N // 128       # 16 key tiles of 128
NQT = NQ // 128      # 8 query tiles of 128
NPAIR = H // 2       # 4 head pairs

# this walrus build encodes at most 1 sync-wait per instruction
_MAX_WAITS = 1


def _split_excess_waits(nc):
    """Move excess per-instruction sem-waits onto preceding NoOps."""
    n_split = 0
    for f in nc.m.functions:
        for blk in f.blocks:
            insts = blk.instructions
            i = 0
            while i < len(insts):
                inst = insts[i]
                si = getattr(inst, "sync_info", None)
                if si is not None and si.on_wait and len(si.on_wait) > _MAX_WAITS:
                    waits = list(si.on_wait)
                    si.on_wait = waits[:_MAX_WAITS]
                    extra = waits[_MAX_WAITS:]
                    new_insts = []
                    for j in range(0, len(extra), _MAX_WAITS):
                        chunk = extra[j : j + _MAX_WAITS]
                        nop = mybir.InstNoOp(
                            name=f"{inst.name}-waitsplit-{j}",
                            engine=inst.engine,
                            ins=[],
                            outs=[],
                            sync_info=mybir.SyncInfo(on_wait=chunk, on_update=[]),
                        )
                        new_insts.append(nop)
                    insts[i:i] = new_insts
                    i += len(new_insts)
                    n_split += 1
                i += 1
    return n_split


def _build():
    nc = bass.Bass()
    # q/k/w arrive pre-transposed (d-major); v arrives as host-packed V_ext.
    qt_d = nc.dram_tensor("qt", [D, NQ], BF16, kind="ExternalInput")
    kt_d = nc.dram_tensor("kt", [D, N], BF16, kind="ExternalInput")
    vx_d = nc.dram_tensor("vx", [128, H * NKT * (HEAD + 1)], F32R, kind="ExternalInput")
    wt_d = nc.dram_tensor("wt", [D, D], F32R, kind="ExternalInput")
    id_d = nc.dram_tensor("ident", [128, 128], F32R, kind="ExternalInput")
    bvec = nc.dram_tensor("bvec", [D], F32, kind="ExternalInput")
    out = nc.dram_tensor("out", [NQ, D], BF16, kind="ExternalOutput")

    vx_r = vx_d[:, :].rearrange("i (h a c) -> i h a c", h=H, a=NKT)
    out_r = out[:, :].rearrange("(a i) d -> i a d", i=128)

    with TileContext(nc) as tc:
        with (
            tc.tile_pool(name="singles", bufs=1) as singles,
            tc.tile_pool(name="tp", bufs=2) as tp,
            tc.tile_pool(name="epool", bufs=16) as epool,
            tc.tile_pool(name="otm", bufs=2) as otm,
            tc.tile_pool(name="psum_s", bufs=2, space="PSUM") as psum_s,
            tc.tile_pool(name="psum_o", bufs=2, space="PSUM") as psum_o,
        ):
            bias_bc = singles.tile([128, D], F32)

            vxt = []    # per-head V_ext tiles [128, 16, 65]
            wts = []    # per-pair packed W^T tiles [128 d_in, 512 d_out]
            fsb = []    # output accumulators [128 q, 512]
            for h in range(H):
                t = singles.tile(
                    [128, NKT, HEAD + 1], F32R, name=f"vxt{h}", tag=f"vxt{h}"
                )
                vxt.append(t)
            for p in range(NPAIR):
                t = singles.tile([128, D], F32R, name=f"wt{p}", tag=f"wt{p}")
                wts.append(t)
            for i in range(NQT):
                t = singles.tile([128, D], F32R, name=f"fsb{i}", tag=f"fsb{i}")
                fsb.append(t)
            ident = singles.tile([128, 128], F32R, name="ident", tag="ident")
            obf = []    # bf16 output staging: halves the final DMA chain
            for j in range(NQT // 2):
                t = singles.tile([128, 2, D], BF16, name=f"obf{j}", tag=f"obf{j}")
                obf.append(t)

            # fp32 ones strip; bitcast views serve as f32r matmul operands
            # (warm-up moving rows and the ones-row broadcast stationary)
            ones_f = singles.tile([1, 512], F32)
            nc.vector.memset(ones_f, 1.0)
            ones_lhs = ones_f[0:1, 0:64].bitcast(F32R)
            ones_rhs = ones_f[0:1, :].bitcast(F32R)
            # selector stationary for the strip-pair reciprocal broadcast:
            # out rows 0:64 take strip row 0 (head A), rows 64:128 take strip
            # row 32 (head B; engine writes must start at partition 0/32/...).
            # Rows 1..31 are zero so the strip tile's unused rows (zeroed
            # once below) contribute nothing.
            sel_f = singles.tile([33, 128], F32)
            nc.vector.memset(sel_f, 0.0)
            nc.vector.memset(sel_f[0:1, 0:64], 1.0)
            nc.vector.memset(sel_f[32:33, 64:128], 1.0)
            sel_lhs = sel_f[:, :].bitcast(F32R)
            st = singles.tile([33, 1024], F32R, name="strips", tag="strips")
            nc.vector.memset(st.bitcast(F32), 0.0)

            def emit_warm(n, tag="s", pool=None):
                """Keep the PE p-state ramped with tiny self-contained mms."""
                pool = pool or psum_s
                for _ in range(n):
                    w_ps = pool.tile([64, 512], F32, name="warm", tag=tag)
                    nc.tensor.matmul(
                        w_ps, lhsT=ones_lhs, rhs=ones_rhs, start=True, stop=True
                    )

            def emit_pair_loads(p, first=False):
                """Issue DMA loads for pair p; returns (qt, kt_sb)."""
                hA, hB = 2 * p, 2 * p + 1
                qt = tp.tile([128, NQ], BF16, name=f"qt{p}", tag="qt")
                kt_sb = tp.tile([128, N], BF16, name=f"ktile{p}", tag="ktile")
                rows = slice(p * 128, (p + 1) * 128)
                if first:
                    # fine-grained startup: unblock the first S matmuls ASAP
                    nc.sync.dma_start(out=qt, in_=qt_d[rows, :])
                    nc.sync.dma_start(out=kt_sb[:, 0:128], in_=kt_d[rows, 0:128])
                    nc.sync.dma_start(out=kt_sb[:, 128:1024], in_=kt_d[rows, 128:1024])
                    nc.sync.dma_start(out=vxt[hA], in_=vx_r[:, hA, :, :])
                    nc.sync.dma_start(out=vxt[hB], in_=vx_r[:, hB, :, :])
                    nc.sync.dma_start(
                        out=kt_sb[:, 1024:2048], in_=kt_d[rows, 1024:2048]
                    )
                else:
                    nc.sync.dma_start(out=qt, in_=qt_d[rows, :])
                    nc.sync.dma_start(out=kt_sb[:, 0:1024], in_=kt_d[rows, 0:1024])
                    nc.sync.dma_start(out=vxt[hA], in_=vx_r[:, hA, :, :])
                    nc.sync.dma_start(
                        out=kt_sb[:, 1024:2048], in_=kt_d[rows, 1024:2048]
                    )
                    nc.sync.dma_start(out=vxt[hB], in_=vx_r[:, hB, :, :])
                nc.sync.dma_start(out=wts[p], in_=wt_d[p * 128 : (p + 1) * 128, :])
                if p == NPAIR - 1:
                    nc.sync.dma_start(out=ident, in_=id_d[:, :])
                return qt, kt_sb

            def emit_norm(p, o_ps, otmp, tail=False):
                """Normalize pair p's O^T pair-tile in place.

                reciprocal(DVE, straight off the PSUM denominator row) ->
                ones-row matmul broadcast (PE, into the freed o slots) ->
                in-place tensor_mul. Drain copies ride DVE+Pool (ACT+Pool
                at the tail, when the exp stream is done).
                """
                # f32r strips: the PE rounds operands anyway; denominators
                # are O(1000) so f32r's mantissa is plenty
                with nc.allow_low_precision(reason="f32r broadcast strip"):
                    nc.vector.reciprocal(st[0:1, :], o_ps[0][64:65, :])
                    nc.vector.reciprocal(st[32:33, :], o_ps[1][64:65, :])
                if tail:
                    # ACT is idle once the exps end: it drains both heads
                    # back-to-back while DVE handles the reciprocals
                    nc.scalar.copy(otmp[0:64, :], o_ps[0][0:64, :])
                    nc.scalar.copy(otmp[64:128, :], o_ps[1][0:64, :])
                else:
                    nc.vector.tensor_copy(otmp[0:64, :], o_ps[0][0:64, :])
                    nc.vector.tensor_copy(otmp[64:128, :], o_ps[1][0:64, :])
                # both heads' reciprocal rows broadcast into ONE [128, 1024]
                # PSUM tile via the selector stationary, so a single
                # tensor_mul normalizes the whole pair tile
                if tail:
                    rbp = psum_s.tile([128, 1024], F32, name=f"rbp{p}", tag="s")
                else:
                    rbp = psum_o.tile([128, 1024], F32, name=f"rbp{p}", tag="o")
                for qc in range(2):
                    nc.tensor.matmul(
                        rbp[:, qc * 512 : (qc + 1) * 512],
                        lhsT=sel_lhs,
                        rhs=st[0:33, qc * 512 : (qc + 1) * 512],
                        start=True,
                        stop=True,
                    )
                # column-chunked so each projection tile (and, at the
                # tail, its output DMA) launches as soon as its q-columns
                # are normalized
                for c in range(4):
                    cols = slice(c * 256, (c + 1) * 256)
                    nc.vector.tensor_mul(
                        otmp[:, cols], otmp[:, cols], rbp[:, cols]
                    )

            def emit_proj(p, otmp):
                """Pair-packed projection of pair p into fsb (+ bias on p0).

                The last pair folds the running fsb into the PSUM group via
                an identity matmul, so the drain is a plain copy that splits
                across the idle ACT and DVE instead of 8 serial DVE adds.
                """
                last = p == NPAIR - 1
                for i in range(NQT):
                    # at the tail the s-slots are free: alternate tags for a
                    # 4-deep proj pipeline instead of 2
                    if last and i % 2 == 1:
                        ps = psum_s.tile([128, 512], F32, name=f"f{p}_{i}", tag="s")
                    else:
                        ps = psum_o.tile([128, 512], F32, name=f"f{p}_{i}", tag="o")
                    nc.tensor.matmul(
                        ps,
                        lhsT=otmp[:, i * 128 : (i + 1) * 128],
                        rhs=wts[p],
                        start=True,
                        stop=not last,
                    )
                    if not last:
                        # fsb is f32r so the last pair's identity matmul can
                        # consume it directly (~1e-4 rounding, within budget)
                        with nc.allow_low_precision(reason="f32r fsb accum"):
                            if p == 0:
                                nc.vector.tensor_add(
                                    out=fsb[i], in0=ps, in1=bias_bc
                                )
                            else:
                                nc.vector.tensor_add(
                                    out=fsb[i], in0=ps, in1=fsb[i]
                                )
                    else:
                        nc.tensor.matmul(
                            ps,
                            lhsT=ident,
                            rhs=fsb[i],
                            start=False,
                            stop=True,
                        )
                        dst = obf[i // 2][:, i % 2, :]
                        if i % 2 == 0:
                            nc.vector.tensor_copy(dst, ps)
                        else:
                            nc.scalar.copy(dst, ps)
                        if i % 2 == 1:
                            j = i // 2
                            nc.sync.dma_start(
                                out=out_r[:, 2 * j : 2 * j + 2, :],
                                in_=obf[j],
                            )

            emit_warm(7)
            nxt = emit_pair_loads(0, first=True)
            for p in range(NPAIR):
                hA, hB = 2 * p, 2 * p + 1
                qt, kt_sb = nxt

                o_ps = {
                    0: psum_o.tile([65, 1024], F32, name=f"o{hA}", tag="o"),
                    1: psum_o.tile([65, 1024], F32, name=f"o{hB}", tag="o"),
                }

                for kt in range(NKT):
                    if p == 0 and kt == 12:
                        nc.sync.dma_start(
                            out=bias_bc, in_=bvec[:].partition_broadcast(128)
                        )
                    if kt == 10 and p + 1 < NPAIR:
                        nxt = emit_pair_loads(p + 1)

                    for hh, h in ((0, hA), (1, hB)):
                        base = hh * 64
                        s_ps = psum_s.tile(
                            [128, 1024], F32, name=f"s{h}_{kt}", tag="s"
                        )
                        # the exp feeders outrank everything else on PE so
                        # the ACT stream never starves behind PV catch-up
                        with tc.high_priority():
                            for qc in range(2):
                                nc.tensor.matmul(
                                    s_ps[:, qc * 512 : (qc + 1) * 512],
                                    lhsT=kt_sb[
                                        base : base + 64, kt * 128 : (kt + 1) * 128
                                    ],
                                    rhs=qt[base : base + 64, qc * 512 : (qc + 1) * 512],
                                    start=True,
                                    stop=True,
                                )
                        e_sb = epool.tile(
                            [128, 1024], F32R, name=f"e{h}_{kt}", tag="e"
                        )
                        nc.scalar.activation(
                            e_sb,
                            s_ps,
                            mybir.ActivationFunctionType.Exp,
                            bias=0.0,
                            scale=1.0 / TEMP,
                        )
                        for qc in range(2):
                            nc.tensor.matmul(
                                o_ps[hh][:, qc * 512 : (qc + 1) * 512],
                                lhsT=vxt[h][:, kt, :],
                                rhs=e_sb[:, qc * 512 : (qc + 1) * 512],
                                start=(kt == 0),
                                stop=(kt == NKT - 1),
                            )

                tail = p == NPAIR - 1
                otmp = otm.tile([128, 1024], F32R, name=f"otmp{p}", tag="ot")
                if tail:
                    emit_warm(8)
                emit_norm(p, o_ps, otmp, tail=tail)
                emit_proj(p, otmp)

    _split_excess_waits(nc)
    return nc


_NC_CACHE = {}


def _get_nc():
    if "nc" not in _NC_CACHE:
        _NC_CACHE["nc"] = _build()
    return _NC_CACHE["nc"]


def kernel(keys, queries, values, W_comb, b_comb, _collect=None):
    from concourse.bass_utils import run_bass_kernel_spmd

    keys = np.ascontiguousarray(keys, dtype=np.float32)
    queries = np.ascontiguousarray(queries, dtype=np.float32)
    values = np.ascontiguousarray(values, dtype=np.float32)
    W_comb = np.ascontiguousarray(W_comb, dtype=np.float32)
    b_comb = np.ascontiguousarray(b_comb, dtype=np.float32)

    nc = _get_nc()
    wt_np = np.ascontiguousarray(W_comb.T)

    # host-packed V_ext per batch: [128, H, 16, 65], ones column baked in
    vx_b = []
    for b in range(B):
        vr = values[b].reshape(NKT, 128, D).transpose(1, 0, 2)  # [i, a, d]
        vx = np.ones((128, H, NKT, HEAD + 1), dtype=np.float32)
        for h in range(H):
            vx[:, h, :, 0:HEAD] = vr[:, :, h * HEAD : (h + 1) * HEAD]
        vx_b.append(np.ascontiguousarray(vx))

    in_maps = []
    for c in range(NCORES):
        b, half = divmod(c, 2)
        in_maps.append(
            {
                "qt": np.ascontiguousarray(
                    queries[b, half * NQ : (half + 1) * NQ, :].T.astype(
                        ml_dtypes.bfloat16
                    )
                ),
                "kt": np.ascontiguousarray(keys[b].T.astype(ml_dtypes.bfloat16)),
                "vx": vx_b[b].reshape(128, -1),
                "wt": wt_np,
                "ident": np.eye(128, dtype=np.float32),
                "bvec": b_comb,
            }
        )
    kwargs = dict(_collect) if _collect else {}
    res = run_bass_kernel_spmd(nc, in_maps, core_ids=list(range(NCORES)), **kwargs)

    full = np.empty((B, N, D), dtype=np.float32)
    for c, r in enumerate(res.results):
        b, half = divmod(c, 2)
        full[b, half * NQ : (half + 1) * NQ, :] = np.asarray(r["out"], dtype=np.float32)
    if _collect is not None:
        return full, res
    return full
